# revision 3
# baseline (speedup 1.0000x reference)
"""Multi-head causal attention on 8 TRN2 NeuronCores — one head per core.

Full inputs in, full output out. Per core (head h):
  Q^T/K^T/V^T = W^T x^T   (PE, bf16)
  S^T[j,i] = K_j . Q_i    (PE, causal-packed, flash-style)
  P^T = exp(S^T/8)        (ScalarE, no max-subtraction: |scores| << 1)
  O^T[v,i] accum += V'[j,(v|1)]^T P^T[j,i]  (PE; row 64 = sumexp)
  out[i,o] = (O^T[:,i]/sumexp_i)^T W_o      (PE + fused row scale on evac)
Host sums the 8 per-head partial outputs.
"""

import numpy as np
import ml_dtypes

import concourse.bass as bass
import concourse.mybir as mybir
import concourse.tile as tile
from concourse import bacc
from concourse.bass_utils import run_bass_kernel_spmd

BF16 = mybir.dt.bfloat16
F32 = mybir.dt.float32

S = 4096
D_IN = 512
D_K = 64
D_V = 64
D_OUT = 512
H = 8
NJT = S // 128   # 32 key tiles
NCH = S // 512   # 8 query chunks
NCK = D_IN // 128  # 4 contraction chunks for projections

_CACHE = {}


def _emit(nc, tc, ctx_pools):
    import contextlib

    xT_d = nc.dram_tensor("xT", [D_IN, S], BF16, kind="ExternalInput").ap()
    wq_d = nc.dram_tensor("wq", [D_IN, D_K], BF16, kind="ExternalInput").ap()
    wk_d = nc.dram_tensor("wk", [D_IN, D_K], BF16, kind="ExternalInput").ap()
    wv_d = nc.dram_tensor("wv", [D_IN, D_V], BF16, kind="ExternalInput").ap()
    wo_d = nc.dram_tensor("wo", [D_V, D_OUT], BF16, kind="ExternalInput").ap()
    mask_d = nc.dram_tensor("mask", [128, 128], BF16, kind="ExternalInput").ap()
    iden_d = nc.dram_tensor("iden", [128, 128], BF16, kind="ExternalInput").ap()
    out_d = nc.dram_tensor("out", [S, D_OUT], F32, kind="ExternalOutput").ap()

    Exp = mybir.ActivationFunctionType.Exp

    with contextlib.ExitStack() as ctx:
        const = ctx.enter_context(tc.tile_pool(name="const", bufs=1))
        persist = ctx.enter_context(tc.tile_pool(name="persist", bufs=1))
        small = ctx.enter_context(tc.tile_pool(name="small", bufs=4))
        outp = ctx.enter_context(tc.tile_pool(name="outp", bufs=4))

        # ---- constants ----
        wq_sb = const.tile([128, NCK * D_K], BF16)   # [128, 256] chunk-major
        wk_sb = const.tile([128, NCK * D_K], BF16)
        wv_sb = const.tile([128, NCK * D_V], BF16)
        wo_sb = const.tile([D_V, D_OUT], BF16)
        mask_sb = const.tile([128, 128], BF16)
        iden_sb = const.tile([128, 128], BF16)
        for c in range(NCK):
            rows = slice(c * 128, (c + 1) * 128)
            nc.sync.dma_start(out=wq_sb[:, c * D_K:(c + 1) * D_K], in_=wq_d[rows, :])
            nc.sync.dma_start(out=wk_sb[:, c * D_K:(c + 1) * D_K], in_=wk_d[rows, :])
            nc.sync.dma_start(out=wv_sb[:, c * D_V:(c + 1) * D_V], in_=wv_d[rows, :])
        nc.sync.dma_start(out=wo_sb, in_=wo_d)
        nc.sync.dma_start(out=mask_sb, in_=mask_d)
        nc.sync.dma_start(out=iden_sb, in_=iden_d)

        # persistent activations
        qt = persist.tile([64, S], BF16)    # Q^T
        kt = persist.tile([64, S], BF16)    # K^T
        vpt = persist.tile([65, S], BF16)   # V'^T: rows 0-63 V^T, row 64 ones
        vp = persist.tile([128, NJT * 65], BF16)  # V' tiles [128, 65] per jt

        nc.vector.memset(vpt[64:65, :], 1.0)

        # ---- stage A: projections Q^T/K^T/V^T = W^T x^T ----
        with tc.tile_pool(name="xt", bufs=1) as xtp, \
             tc.tile_pool(name="psA", bufs=4, space="PSUM") as psA:
            xts = []
            for c in range(NCK):
                xt = xtp.tile([128, S], BF16, tag=f"xt{c}")
                nc.sync.dma_start(out=xt, in_=xT_d[c * 128:(c + 1) * 128, :])
                xts.append(xt)
            for st in range(NCH):
                sl = bass.ts(st, 512)
                for w_sb, dest, dcopy in (
                    (wq_sb, qt, nc.vector.tensor_copy),
                    (wk_sb, kt, nc.vector.tensor_copy),
                    (wv_sb, vpt[0:64, :], nc.scalar.copy),
                ):
                    ps = psA.tile([64, 512], F32, tag="psA")
                    for c in range(NCK):
                        nc.tensor.matmul(
                            ps,
                            lhsT=w_sb[:, c * 64:(c + 1) * 64],
                            rhs=xts[c][:, sl],
                            start=(c == 0),
                            stop=(c == NCK - 1),
                        )
                    dcopy(dest[:, sl], ps)

            # ---- stage A2: V' tiles via PE transpose ----
            with tc.tile_pool(name="psT", bufs=4, space="PSUM") as psT:
                for jt in range(NJT):
                    pst = psT.tile([128, 65], BF16, tag="psT")
                    nc.tensor.transpose(
                        pst,
                        vpt[:, jt * 128:(jt + 1) * 128],
                        iden_sb[0:65, 0:65],
                    )
                    nc.vector.tensor_copy(vp[:, jt * 65:(jt + 1) * 65], pst)

        # ---- pass 1: S^T matmuls + exp -> causal-packed P^T ----
        pt_pool = ctx.enter_context(tc.tile_pool(name="pt", bufs=1))
        pts = []
        with tc.tile_pool(name="psB", bufs=2, space="PSUM") as psB:
            for jt in range(NJT):
                i0 = jt * 128           # diagonal start
                c0 = jt // 4            # first query chunk
                pt = pt_pool.tile([128, S - i0], BF16, tag=f"pt{jt}")
                pts.append(pt)
                ktile = kt[:, jt * 128:(jt + 1) * 128]
                for g0 in range(c0, NCH, 4):
                    g1 = min(g0 + 4, NCH)
                    ps = psB.tile([128, 2048], F32, tag="psB")
                    for c in range(g0, g1):
                        lo = max(c * 512, i0)
                        hi = (c + 1) * 512
                        nc.tensor.matmul(
                            ps[:, (c - g0) * 512 + lo - c * 512:
                                  (c - g0) * 512 + hi - c * 512],
                            lhsT=ktile,
                            rhs=qt[:, lo:hi],
                            start=True,
                            stop=True,
                        )
                    glo = max(g0 * 512, i0)
                    ghi = g1 * 512
                    nc.scalar.activation(
                        pt[:, glo - i0:ghi - i0],
                        ps[:, glo - g0 * 512:ghi - g0 * 512],
                        Exp,
                        scale=0.125,
                    )
                # causal mask on the diagonal 128x128 block
                nc.vector.tensor_mul(pt[:, 0:128], pt[:, 0:128], mask_sb)

        # ---- pass 2: O^T accumulation + progressive output projection ----
        def out_proj(c, ot_bf, rcols):
            for ib in range(4):
                po = psC.tile([128, 512], F32, tag="bank")
                nc.tensor.matmul(
                    po,
                    lhsT=ot_bf[0:64, ib * 128:(ib + 1) * 128],
                    rhs=wo_sb,
                    start=True,
                    stop=True,
                )
                ob = outp.tile([128, 512], F32, tag="ob")
                nc.vector.tensor_scalar_mul(ob, po, rcols[:, ib:ib + 1])
                nc.sync.dma_start(
                    out=out_d[c * 512 + ib * 128:c * 512 + (ib + 1) * 128, :],
                    in_=ob,
                )

        with tc.tile_pool(name="psC", bufs=8, space="PSUM") as psC:
            accs = [psC.tile([65, 512], F32, tag="bank", name=f"acc{i}") for i in range(NCH)]
            pending = []
            for jt in range(NJT):
                c0 = jt // 4
                i0 = jt * 128
                # emit deferred out-projection first (overlaps with accums)
                if pending:
                    out_proj(*pending.pop())
                for c in range(c0, NCH):
                    lo = max(c * 512, i0)
                    hi = (c + 1) * 512
                    nc.tensor.matmul(
                        accs[c][:, lo - c * 512:hi - c * 512],
                        lhsT=vp[:, jt * 65:(jt + 1) * 65],
                        rhs=pts[jt][:, lo - i0:hi - i0],
                        start=(jt == 0),
                        stop=(jt == 4 * c + 3),
                    )
                if jt % 4 == 3:
                    c = jt // 4
                    # evacuate finished accumulator c (DVE/DMA only)
                    ot_bf = small.tile([65, 512], BF16, tag="otbf")
                    nc.vector.tensor_copy(ot_bf, accs[c])
                    rs = small.tile([1, 512], F32, tag="rs")
                    nc.vector.reciprocal(rs, ot_bf[64:65, :])
                    rcols = small.tile([128, 4], F32, tag="rcols")
                    for ib in range(4):
                        nc.sync.dma_start(
                            out=rcols[:, ib:ib + 1],
                            in_=rs[0:1, ib * 128:(ib + 1) * 128],
                        )
                    pending.append((c, ot_bf, rcols))
            while pending:
                out_proj(*pending.pop())


def _build():
    if "nc" in _CACHE:
        return _CACHE["nc"]
    nc = bacc.Bacc("TRN2", target_bir_lowering=False, debug=False)
    with tile.TileContext(nc) as tc:
        _emit(nc, tc, None)
    nc.compile()
    _CACHE["nc"] = nc
    return nc


def kernel(x, W_q, W_k, W_v, W_o):
    nc = _build()
    bf = ml_dtypes.bfloat16
    xT = np.ascontiguousarray(x.reshape(S, D_IN).T).astype(bf)
    mask = np.triu(np.ones((128, 128), np.float32)).astype(bf)
    iden = np.eye(128, dtype=np.float32).astype(bf)
    in_maps = []
    for h in range(H):
        in_maps.append({
            "xT": xT,
            "wq": np.ascontiguousarray(W_q[h]).astype(bf),
            "wk": np.ascontiguousarray(W_k[h]).astype(bf),
            "wv": np.ascontiguousarray(W_v[h]).astype(bf),
            "wo": np.ascontiguousarray(W_o[h]).astype(bf),
            "mask": mask,
            "iden": iden,
        })
    res = run_bass_kernel_spmd(nc, in_maps, core_ids=list(range(H)))
    out = np.zeros((S, D_OUT), np.float32)
    for h in range(H):
        out += res.results[h]["out"]
    return out[None]
